# revision 18
# baseline (speedup 1.0000x reference)
"""DenseAttentionAggregator Trainium2 kernel (8-core SPMD).

v2: neigh-side projections sharded across cores + AllGather; fused masked
attention with chunked pipeline; mask applied by multiply-after-exp with the
row-sum obtained from an extra ones-column in the value matrix.

Sharding: rows of x/adj (data parallel) and rows of neigh (projection stage)
split across 8 cores. Each core gets x[c], adj[c], neigh[c] slices; value and
neib_att are all-gathered on-device.
"""

import sys

sys.path.insert(0, "/opt/trn_rl_repo")

import numpy as np

import concourse.bass as bass
import concourse.bacc as bacc
import concourse.tile as tile
from concourse import mybir
from concourse.bass import ts, ds
from concourse.bass_utils import run_bass_kernel_spmd
from concourse.masks import make_identity
from concourse.tile_rust import add_dep_helper

N_X, N_NEIGH, D_IN, H, D_OUT = 8192, 8192, 512, 32, 256
N_CORES = 8
R = N_X // N_CORES            # rows per core (1024)
P = 128
NB = R // P                   # x row-blocks per core (8)
NTL = R // P                  # local neigh tiles per core (8)
NT = N_NEIGH // P             # total neigh tiles (64)
KC = D_IN // P                # contraction chunks (4)
VW = D_OUT + 1                # value width with ones column (257)
TB = 8                        # transposes per PSUM batch

f32 = mybir.dt.float32
bf16 = mybir.dt.bfloat16
i32 = mybir.dt.int32
AF = mybir.ActivationFunctionType
OP = mybir.AluOpType

_CACHE = {}


def _build():
    nc = bacc.Bacc("TRN2", target_bir_lowering=False, debug=False,
                   num_devices=N_CORES)

    def din(name, shape, dt):
        return nc.dram_tensor(name, list(shape), dt, kind="ExternalInput").ap()

    x_in = din("x", (R, D_IN), f32)
    ng_in = din("neigh_slice", (R, D_IN), f32)
    adj_in = din("adj", (R, N_NEIGH), i32)
    wn1_in = din("wn1", (P, KC, H), f32)
    wx1_in = din("wx1", (P, KC, H), f32)
    wn2_in = din("wn2", (H, H), f32)
    wx2_in = din("wx2", (H, H), f32)
    wv_in = din("wv", (P, KC, D_OUT), f32)
    wfx_in = din("wfx", (P, KC, D_OUT), f32)
    bn1_in = din("bn1", (H, 1), f32)
    bx1_in = din("bx1", (H, 1), f32)
    bn2_in = din("bn2", (H, 1), f32)
    bx2_in = din("bx2", (H, 1), f32)
    bvb_in = din("bvb", (P, D_OUT), f32)
    bfxb_in = din("bfxb", (P, D_OUT), f32)

    out_ap = nc.dram_tensor("out", [R, 2 * D_OUT], f32, kind="ExternalOutput").ap()

    # collective bounce buffers
    vb_in = nc.dram_tensor("vb_in", [NTL, P, D_OUT], bf16)
    vb_gat = nc.dram_tensor("vb_gat", [NT, P, D_OUT], bf16, addr_space="Shared")
    ab_in = nc.dram_tensor("ab_in", [H, R], bf16)
    ab_gat = nc.dram_tensor("ab_gat", [N_CORES * H, R], bf16, addr_space="Shared")

    with tile.TileContext(nc) as tc:
        with (
            tc.tile_pool(name="consts", bufs=1) as consts,
            tc.tile_pool(name="work", bufs=3) as work,
            tc.tile_pool(name="blk", bufs=2) as blk,
            tc.tile_pool(name="pws", bufs=2, space="PSUM") as pws,
            tc.tile_pool(name="ppt", bufs=2, space="PSUM") as ppt,
            tc.tile_pool(name="pacc", bufs=2, space="PSUM") as pacc,
        ):
            # ---------------- constants ----------------
            ident_bf = consts.tile([P, P], bf16)
            make_identity(nc, ident_bf[:])
            ident_f = consts.tile([P, P], f32)
            make_identity(nc, ident_f[:])

            wn1_bf = consts.tile([P, KC, H], bf16)
            nc.gpsimd.dma_start(out=wn1_bf[:], in_=wn1_in[:])
            wn2_bf = consts.tile([H, H], bf16)
            nc.gpsimd.dma_start(out=wn2_bf[:], in_=wn2_in[:])
            wv_bf = consts.tile([P, KC, D_OUT], bf16)
            nc.gpsimd.dma_start(out=wv_bf[:], in_=wv_in[:])
            wx1_f = consts.tile([P, KC, H], f32)
            nc.sync.dma_start(out=wx1_f[:], in_=wx1_in[:])
            wx2_f = consts.tile([H, H], f32)
            nc.sync.dma_start(out=wx2_f[:], in_=wx2_in[:])
            wfx_f = consts.tile([P, KC, D_OUT], f32)
            nc.sync.dma_start(out=wfx_f[:], in_=wfx_in[:])

            def bias_tile(src, parts, tag):
                t = consts.tile([parts, 1], f32, tag=tag)
                nc.sync.dma_start(out=t[:], in_=src[:])
                return t
            bn1_t = bias_tile(bn1_in, H, "bn1t")
            bx1_t = bias_tile(bx1_in, H, "bx1t")
            bn2_t = bias_tile(bn2_in, H, "bn2t")
            bx2_t = bias_tile(bx2_in, H, "bx2t")
            bvb_t = consts.tile([P, D_OUT], f32)
            nc.sync.dma_start(out=bvb_t[:], in_=bvb_in[:])
            bfxb_t = consts.tile([P, D_OUT], f32)
            nc.sync.dma_start(out=bfxb_t[:], in_=bfxb_in[:])

            value_all = consts.tile([P, NT, VW], bf16)       # ~32 KB/part
            nattT = consts.tile([H, N_CORES, R], bf16)       # 16 KB
            xattT = consts.tile([H, R], bf16)

            # ---------------- phase 1: local neigh projections ----------------
            ng_big = consts.tile([P, NTL, D_IN], bf16)       # 8 KB/part
            # split: a single (p, 8, d) SWDGE cast DMA needs 1024 descriptors,
            # which fills the whole SWDGE ring and deadlocks
            ng_r = ng_in.rearrange("(t p) d -> p t d", p=P)
            nc.gpsimd.dma_start(out=ng_big[:, 0:NTL // 2, :],
                                in_=ng_r[:, 0:NTL // 2, :])
            nc.gpsimd.dma_start(out=ng_big[:, NTL // 2:NTL, :],
                                in_=ng_r[:, NTL // 2:NTL, :])
            vslice = consts.tile([P, NTL, D_OUT], bf16)
            attT_sl = consts.tile([H, R], bf16)

            for jp in range(NTL // 2):      # pairs of neigh tiles
                ntT_ps = pws.tile([P, KC, 2 * P], bf16, tag="pws")
                for t in range(2):
                    for k in range(KC):
                        nc.tensor.transpose(ntT_ps[:, k, ts(t, P)],
                                            ng_big[:, 2 * jp + t, ts(k, P)],
                                            ident_bf[:])
                ntT = work.tile([P, KC, 2 * P], bf16, tag="ntT")
                nc.any.tensor_copy(ntT[:], ntT_ps[:])

                for t in range(2):
                    v_ps = pacc.tile([P, D_OUT], f32, tag="pacc")
                    for k in range(KC):
                        nc.tensor.matmul(v_ps[:], ntT[:, k, ts(t, P)],
                                         wv_bf[:, k, :],
                                         start=(k == 0), stop=(k == KC - 1))
                    nc.any.tensor_copy(vslice[:, 2 * jp + t, :], v_ps[:])

                u_ps = pacc.tile([H, 2 * P], f32, tag="pacc")
                for k in range(KC):
                    nc.tensor.matmul(u_ps[:], wn1_bf[:, k, :], ntT[:, k, :],
                                     start=(k == 0), stop=(k == KC - 1))
                tT = work.tile([H, 2 * P], bf16, tag="tT")
                nc.scalar.activation(tT[:], u_ps[:], AF.Tanh, bias=bn1_t[:, 0:1])

                a_ps = pacc.tile([H, 2 * P], f32, tag="pacc")
                nc.tensor.matmul(a_ps[:], wn2_bf[:], tT[:], start=True, stop=True)
                nc.any.tensor_scalar(attT_sl[:, ts(jp, 2 * P)], a_ps[:],
                                     bn2_t[:, 0:1], None, OP.add)

            # attT all-gather first: it unblocks the score matmuls.
            # nattT reload rides the scalar-engine HWDGE queue (ahead of the
            # prelus, behind nothing slow); value reloads go on the SP queue
            # *after* phase 2 so they don't head-of-line-block x loads.
            nc.sync.dma_start(out=ab_in[:], in_=attT_sl[:])
            cc_att = nc.gpsimd.collective_compute(
                "AllGather", OP.bypass, replica_groups=[list(range(N_CORES))],
                ins=[ab_in[:]], outs=[ab_gat[:]])
            nc.scalar.dma_start(out=nattT[:],
                                in_=ab_gat.ap().rearrange("(c h) r -> h c r", h=H))

            # prefetch the first two adj blocks (pool bufs=2) ahead of the
            # value collective so their DMAs overlap the gather phase
            adjf_list = {}
            for b in range(2):
                adjf_list[b] = blk.tile([P, N_NEIGH], bf16, tag="adjf",
                                        name=f"adjf{b}")
                nc.gpsimd.dma_start(out=adjf_list[b][:], in_=adj_in[ts(b, P), :])

            vb_r = vb_in.ap().rearrange("t p o -> p t o")
            nc.sync.dma_start(out=vb_r[:, 0:NTL // 2, :],
                              in_=vslice[:, 0:NTL // 2, :])
            nc.sync.dma_start(out=vb_r[:, NTL // 2:NTL, :],
                              in_=vslice[:, NTL // 2:NTL, :])
            cc_val = nc.gpsimd.collective_compute(
                "AllGather", OP.bypass, replica_groups=[list(range(N_CORES))],
                ins=[vb_in[:]], outs=[vb_gat[:]])
            # the big value gather must not jump ahead of the small attT
            # gather (it gates the first score matmuls)
            add_dep_helper(cc_val.ins, cc_att.ins, False,
                           "attT collective first")

            # ---------------- phase 2: x projections + fc_x ----------------
            x_big = consts.tile([P, NB, D_IN], f32)          # 16 KB/part
            x_r = x_in.rearrange("(t p) d -> p t d", p=P)
            nc.sync.dma_start(out=x_big[:, 0:NB // 2, :], in_=x_r[:, 0:NB // 2, :])
            nc.sync.dma_start(out=x_big[:, NB // 2:NB, :], in_=x_r[:, NB // 2:NB, :])
            for i in range(NB):
                xT_ps = pws.tile([P, KC, P], f32, tag="pws")
                for k in range(KC):
                    nc.tensor.transpose(xT_ps[:, k, :], x_big[:, i, ts(k, P)],
                                        ident_f[:])
                xT = work.tile([P, KC, P], f32, tag="xT")
                nc.any.tensor_copy(xT[:], xT_ps[:])

                f_ps = pacc.tile([P, D_OUT], f32, tag="pacc")
                for k in range(KC):
                    nc.tensor.matmul(f_ps[:], xT[:, k, :], wfx_f[:, k, :],
                                     start=(k == 0), stop=(k == KC - 1))
                out1 = work.tile([P, D_OUT], f32, tag="out1")
                nc.vector.tensor_tensor(out1[:], f_ps[:], bfxb_t[:], OP.add)
                nc.sync.dma_start(out=out_ap[ts(i, P), 0:D_OUT], in_=out1[:])

                xu_ps = pacc.tile([H, P], f32, tag="pacc")
                for k in range(KC):
                    nc.tensor.matmul(xu_ps[:], wx1_f[:, k, :], xT[:, k, :],
                                     start=(k == 0), stop=(k == KC - 1))
                txT = work.tile([H, P], f32, tag="txT")
                nc.scalar.activation(txT[:], xu_ps[:], AF.Tanh, bias=bx1_t[:, 0:1])

                xa_ps = pacc.tile([H, P], f32, tag="pacc")
                nc.tensor.matmul(xa_ps[:], wx2_f[:], txT[:], start=True, stop=True)
                nc.any.tensor_scalar(xattT[:, ts(i, P)], xa_ps[:], bx2_t[:, 0:1],
                                     None, OP.add)

            # value reload after phase 2: waits on the collective anyway, and
            # only the (late) out2 writes sit behind it on the SP queue
            vg_r = vb_gat.ap().rearrange("t p o -> p t o")
            for g in range(NT // 4):
                nc.sync.dma_start(out=value_all[:, ts(g, 4), 0:D_OUT],
                                  in_=vg_r[:, ts(g, 4), :])
            nc.vector.memset(value_all[:, :, D_OUT:VW], 1.0)

            # ---------------- phase 3: fused masked attention ----------------
            for b in range(NB):
                adjf = adjf_list.pop(b)
                if b + 2 < NB:
                    adjf_list[b + 2] = blk.tile([P, N_NEIGH], bf16, tag="adjf",
                                            name=f"adjf{b+2}")
                    nc.gpsimd.dma_start(out=adjf_list[b + 2][:],
                                        in_=adj_in[ts(b + 2, P), :])
                scores = blk.tile([P, N_NEIGH], bf16, tag="scores")
                agg_ps = pacc.tile([P, VW], f32, tag="pacc")

                for c4 in range(4):            # 2048-wide chunks
                    for q in range(2):         # 1024-wide prelu chunks
                        ch = c4 * 2 + q        # 1024-chunk index (0..7)
                        s_ps = pws.tile([P, 1024], f32, tag="pws")
                        for hh in range(2):
                            c = ch * 2 + hh    # 512-chunk index (0..15)
                            nc.tensor.matmul(
                                s_ps[:, ts(hh, 512)], xattT[:, ts(b, P)],
                                nattT[:, c // 2, ds((c % 2) * 512, 512)],
                                start=True, stop=True)
                        nc.scalar.activation(scores[:, ts(ch, 1024)], s_ps[:],
                                             AF.Prelu, alpha=0.01)
                    nc.scalar.activation(scores[:, ts(c4, 2048)],
                                         scores[:, ts(c4, 2048)], AF.Exp)
                    nc.vector.tensor_tensor(scores[:, ts(c4, 2048)],
                                            scores[:, ts(c4, 2048)],
                                            adjf[:, ts(c4, 2048)], OP.mult)
                    for h8 in range(2048 // (TB * P)):
                        pT_ps = ppt.tile([P, TB, P], bf16, tag="ppt")
                        for t in range(TB):
                            c2 = c4 * 16 + h8 * TB + t
                            nc.tensor.transpose(pT_ps[:, t, :],
                                                scores[:, ts(c2, P)], ident_bf[:])
                        pT = work.tile([P, TB, P], bf16, tag="pT")
                        nc.vector.tensor_copy(pT[:], pT_ps[:])
                        for t in range(TB):
                            c2 = c4 * 16 + h8 * TB + t
                            nc.tensor.matmul(agg_ps[:], pT[:, t, :],
                                             value_all[:, c2, :],
                                             start=(c2 == 0), stop=(c2 == NT - 1))

                rcp = work.tile([P, 1], f32, tag="rcp")
                nc.vector.reciprocal(rcp[:], agg_ps[:, D_OUT:VW])
                out2 = work.tile([P, D_OUT], f32, tag="out2")
                nc.vector.tensor_scalar(out2[:], agg_ps[:, 0:D_OUT], rcp[:, 0:1],
                                        None, OP.mult)
                nc.vector.tensor_tensor(out2[:], out2[:], bvb_t[:], OP.add)
                nc.sync.dma_start(out=out_ap[ts(b, P), D_OUT:2 * D_OUT], in_=out2[:])

    nc.compile()
    return nc


def _get_nc():
    if "nc" not in _CACHE:
        _CACHE["nc"] = _build()
    return _CACHE["nc"]


def _prep_weights(Wx1, bx1, Wx2, bx2, Wn1, bn1, Wn2, bn2, Wv, bv, Wfx, bfx):
    def kstripe(w):  # (512, M) -> (128, 4, M) with ki on partitions
        m = w.shape[1]
        return np.ascontiguousarray(
            w.reshape(KC, P, m).transpose(1, 0, 2)).astype(np.float32)

    return {
        "wn1": kstripe(Wn1), "wx1": kstripe(Wx1),
        "wn2": np.ascontiguousarray(Wn2, np.float32),
        "wx2": np.ascontiguousarray(Wx2, np.float32),
        "wv": kstripe(Wv), "wfx": kstripe(Wfx),
        "bn1": np.ascontiguousarray(bn1, np.float32).reshape(H, 1),
        "bx1": np.ascontiguousarray(bx1, np.float32).reshape(H, 1),
        "bn2": np.ascontiguousarray(bn2, np.float32).reshape(H, 1),
        "bx2": np.ascontiguousarray(bx2, np.float32).reshape(H, 1),
        "bvb": np.ascontiguousarray(np.broadcast_to(bv, (P, D_OUT)), np.float32),
        "bfxb": np.ascontiguousarray(np.broadcast_to(bfx, (P, D_OUT)), np.float32),
    }


def make_in_maps(inputs):
    x = np.asarray(inputs["x"], np.float32)
    neigh = np.asarray(inputs["neigh"], np.float32)
    adj = np.asarray(inputs["adj"], np.int32)
    wmap = _prep_weights(*(np.asarray(inputs[k]) for k in
                           ("Wx1", "bx1", "Wx2", "bx2", "Wn1", "bn1", "Wn2", "bn2",
                            "Wv", "bv", "Wfx", "bfx")))
    in_maps = []
    for c in range(N_CORES):
        m = {"x": np.ascontiguousarray(x[c * R:(c + 1) * R]),
             "neigh_slice": np.ascontiguousarray(neigh[c * R:(c + 1) * R]),
             "adj": np.ascontiguousarray(adj[c * R:(c + 1) * R])}
        m.update(wmap)
        in_maps.append(m)
    return in_maps


def run(inputs, trace=False):
    nc = _get_nc()
    in_maps = make_in_maps(inputs)
    res = run_bass_kernel_spmd(nc, in_maps, list(range(N_CORES)), trace=trace)
    out = np.concatenate([res.results[c]["out"] for c in range(N_CORES)], axis=0)
    return out, res


def kernel(**inputs) -> np.ndarray:
    out, _ = run(inputs)
    return out


# revision 20
# speedup vs baseline: 3.3802x; 3.3802x over previous
"""DenseAttentionAggregator Trainium2 kernel (8-core SPMD).

v2: neigh-side projections sharded across cores + AllGather; fused masked
attention with chunked pipeline; mask applied by multiply-after-exp with the
row-sum obtained from an extra ones-column in the value matrix.

Sharding: rows of x/adj (data parallel) and rows of neigh (projection stage)
split across 8 cores. Each core gets x[c], adj[c], neigh[c] slices; value and
neib_att are all-gathered on-device.
"""

import sys

sys.path.insert(0, "/opt/trn_rl_repo")

import numpy as np

import concourse.bass as bass
import concourse.bacc as bacc
import concourse.tile as tile
from concourse import mybir
from concourse.bass import ts, ds
from concourse.bass_utils import run_bass_kernel_spmd
from concourse.masks import make_identity
from concourse.tile_rust import add_dep_helper

N_X, N_NEIGH, D_IN, H, D_OUT = 8192, 8192, 512, 32, 256
N_CORES = 8
R = N_X // N_CORES            # rows per core (1024)
P = 128
NB = R // P                   # x row-blocks per core (8)
NTL = R // P                  # local neigh tiles per core (8)
NT = N_NEIGH // P             # total neigh tiles (64)
KC = D_IN // P                # contraction chunks (4)
VW = D_OUT + 1                # value width with ones column (257)
TB = 8                        # transposes per PSUM batch

f32 = mybir.dt.float32
bf16 = mybir.dt.bfloat16
i32 = mybir.dt.int32
AF = mybir.ActivationFunctionType
OP = mybir.AluOpType

_CACHE = {}


def _build():
    nc = bacc.Bacc("TRN2", target_bir_lowering=False, debug=False,
                   num_devices=N_CORES)

    def din(name, shape, dt):
        return nc.dram_tensor(name, list(shape), dt, kind="ExternalInput").ap()

    x_in = din("x", (R, D_IN), f32)
    ng_in = din("neigh_slice", (R, D_IN), f32)
    adj_in = din("adj", (R, N_NEIGH), i32)
    wn1_in = din("wn1", (P, KC, H), f32)
    wx1_in = din("wx1", (P, KC, H), f32)
    wn2_in = din("wn2", (H, H), f32)
    wx2_in = din("wx2", (H, H), f32)
    wv_in = din("wv", (P, KC, D_OUT), f32)
    wfx_in = din("wfx", (P, KC, D_OUT), f32)
    bn1_in = din("bn1", (H, 1), f32)
    bx1_in = din("bx1", (H, 1), f32)
    bn2_in = din("bn2", (H, 1), f32)
    bx2_in = din("bx2", (H, 1), f32)
    bvb_in = din("bvb", (P, D_OUT), f32)
    bfxb_in = din("bfxb", (P, D_OUT), f32)

    out_ap = nc.dram_tensor("out", [R, 2 * D_OUT], f32, kind="ExternalOutput").ap()

    # collective bounce buffers
    vb_in = nc.dram_tensor("vb_in", [NTL, P, D_OUT], bf16)
    vb_gats = [nc.dram_tensor(f"vb_gat{g}", [N_CORES * 2, P, D_OUT], bf16,
                              addr_space="Shared") for g in range(4)]
    ab_in = nc.dram_tensor("ab_in", [H, R], bf16)
    ab_gat = nc.dram_tensor("ab_gat", [N_CORES * H, R], bf16, addr_space="Shared")

    with tile.TileContext(nc) as tc:
        with (
            tc.tile_pool(name="consts", bufs=1) as consts,
            tc.tile_pool(name="work", bufs=3) as work,
            tc.tile_pool(name="blk", bufs=2) as blk,
            tc.tile_pool(name="pws", bufs=2, space="PSUM") as pws,
            tc.tile_pool(name="ppt", bufs=2, space="PSUM") as ppt,
            tc.tile_pool(name="pacc", bufs=2, space="PSUM") as pacc,
        ):
            # ---------------- constants ----------------
            ident_bf = consts.tile([P, P], bf16)
            make_identity(nc, ident_bf[:])
            ident_f = consts.tile([P, P], f32)
            make_identity(nc, ident_f[:])

            wn1_bf = consts.tile([P, KC, H], bf16)
            nc.gpsimd.dma_start(out=wn1_bf[:], in_=wn1_in[:])
            wn2_bf = consts.tile([H, H], bf16)
            nc.gpsimd.dma_start(out=wn2_bf[:], in_=wn2_in[:])
            wv_bf = consts.tile([P, KC, D_OUT], bf16)
            nc.gpsimd.dma_start(out=wv_bf[:], in_=wv_in[:])
            wx1_f = consts.tile([P, KC, H], f32)
            nc.sync.dma_start(out=wx1_f[:], in_=wx1_in[:])
            wx2_f = consts.tile([H, H], f32)
            nc.sync.dma_start(out=wx2_f[:], in_=wx2_in[:])
            wfx_f = consts.tile([P, KC, D_OUT], f32)
            nc.sync.dma_start(out=wfx_f[:], in_=wfx_in[:])

            def bias_tile(src, parts, tag):
                t = consts.tile([parts, 1], f32, tag=tag)
                nc.sync.dma_start(out=t[:], in_=src[:])
                return t
            bn1_t = bias_tile(bn1_in, H, "bn1t")
            bx1_t = bias_tile(bx1_in, H, "bx1t")
            bn2_t = bias_tile(bn2_in, H, "bn2t")
            bx2_t = bias_tile(bx2_in, H, "bx2t")
            bvb_t = consts.tile([P, D_OUT], f32)
            nc.sync.dma_start(out=bvb_t[:], in_=bvb_in[:])
            bfxb_t = consts.tile([P, D_OUT], f32)
            nc.sync.dma_start(out=bfxb_t[:], in_=bfxb_in[:])

            value_all = consts.tile([P, NT, VW], bf16)       # ~32 KB/part
            nc.vector.memset(value_all[:, :, D_OUT:VW], 1.0)
            nattT = consts.tile([H, N_CORES, R], bf16)       # 16 KB
            xattT = consts.tile([H, R], bf16)

            # ---------------- phase 1: local neigh projections ----------------
            ng_big = consts.tile([P, NTL, D_IN], bf16)       # 8 KB/part
            # split: a single (p, 8, d) SWDGE cast DMA needs 1024 descriptors,
            # which fills the whole SWDGE ring and deadlocks
            ng_r = ng_in.rearrange("(t p) d -> p t d", p=P)
            nc.gpsimd.dma_start(out=ng_big[:, 0:NTL // 2, :],
                                in_=ng_r[:, 0:NTL // 2, :])
            nc.gpsimd.dma_start(out=ng_big[:, NTL // 2:NTL, :],
                                in_=ng_r[:, NTL // 2:NTL, :])
            vslice = consts.tile([P, NTL, D_OUT], bf16)
            attT_sl = consts.tile([H, R], bf16)

            for jp in range(NTL // 2):      # pairs of neigh tiles
                ntT_ps = pws.tile([P, KC, 2 * P], bf16, tag="pws")
                for t in range(2):
                    for k in range(KC):
                        nc.tensor.transpose(ntT_ps[:, k, ts(t, P)],
                                            ng_big[:, 2 * jp + t, ts(k, P)],
                                            ident_bf[:])
                ntT = work.tile([P, KC, 2 * P], bf16, tag="ntT")
                nc.any.tensor_copy(ntT[:], ntT_ps[:])

                for t in range(2):
                    v_ps = pacc.tile([P, D_OUT], f32, tag="pacc")
                    for k in range(KC):
                        nc.tensor.matmul(v_ps[:], ntT[:, k, ts(t, P)],
                                         wv_bf[:, k, :],
                                         start=(k == 0), stop=(k == KC - 1))
                    nc.any.tensor_copy(vslice[:, 2 * jp + t, :], v_ps[:])

                u_ps = pacc.tile([H, 2 * P], f32, tag="pacc")
                for k in range(KC):
                    nc.tensor.matmul(u_ps[:], wn1_bf[:, k, :], ntT[:, k, :],
                                     start=(k == 0), stop=(k == KC - 1))
                tT = work.tile([H, 2 * P], bf16, tag="tT")
                nc.scalar.activation(tT[:], u_ps[:], AF.Tanh, bias=bn1_t[:, 0:1])

                a_ps = pacc.tile([H, 2 * P], f32, tag="pacc")
                nc.tensor.matmul(a_ps[:], wn2_bf[:], tT[:], start=True, stop=True)
                nc.any.tensor_scalar(attT_sl[:, ts(jp, 2 * P)], a_ps[:],
                                     bn2_t[:, 0:1], None, OP.add)

            # attT all-gather first: it unblocks the score matmuls.
            # nattT reload rides the scalar-engine HWDGE queue (ahead of the
            # prelus, behind nothing slow); value reloads go on the SP queue
            # *after* phase 2 so they don't head-of-line-block x loads.
            nc.sync.dma_start(out=ab_in[:], in_=attT_sl[:])
            cc_att = nc.gpsimd.collective_compute(
                "AllGather", OP.bypass, replica_groups=[list(range(N_CORES))],
                ins=[ab_in[:]], outs=[ab_gat[:]])
            nc.scalar.dma_start(out=nattT[:],
                                in_=ab_gat.ap().rearrange("(c h) r -> h c r", h=H))

            # prefetch the first two adj blocks (pool bufs=2) ahead of the
            # value collective so their DMAs overlap the gather phase
            adjf_list = {}
            for b in range(2):
                adjf_list[b] = blk.tile([P, N_NEIGH], bf16, tag="adjf",
                                        name=f"adjf{b}")
                nc.gpsimd.dma_start(out=adjf_list[b][:], in_=adj_in[ts(b, P), :])

            vb_r = vb_in.ap().rearrange("t p o -> p t o")
            nc.sync.dma_start(out=vb_r[:, 0:NTL // 2, :],
                              in_=vslice[:, 0:NTL // 2, :])
            nc.sync.dma_start(out=vb_r[:, NTL // 2:NTL, :],
                              in_=vslice[:, NTL // 2:NTL, :])
            # value gather in 4 chunks (2 local tiles each) so the first agg
            # matmuls unblock as soon as chunk 0 lands; chain them behind the
            # attT gather which gates the score matmuls
            cc_prev = cc_att
            for g in range(4):
                cc_g = nc.gpsimd.collective_compute(
                    "AllGather", OP.bypass,
                    replica_groups=[list(range(N_CORES))],
                    ins=[vb_in[2 * g:2 * g + 2]], outs=[vb_gats[g][:]])
                add_dep_helper(cc_g.ins, cc_prev.ins, False,
                               "collective chunk order")
                cc_prev = cc_g

            # ---------------- phase 2: x projections + fc_x ----------------
            x_big = consts.tile([P, NB, D_IN], f32)          # 16 KB/part
            x_r = x_in.rearrange("(t p) d -> p t d", p=P)
            nc.sync.dma_start(out=x_big[:, 0:NB // 2, :], in_=x_r[:, 0:NB // 2, :])
            nc.sync.dma_start(out=x_big[:, NB // 2:NB, :], in_=x_r[:, NB // 2:NB, :])
            for i in range(NB):
                xT_ps = pws.tile([P, KC, P], f32, tag="pws")
                for k in range(KC):
                    nc.tensor.transpose(xT_ps[:, k, :], x_big[:, i, ts(k, P)],
                                        ident_f[:])
                xT = work.tile([P, KC, P], f32, tag="xT")
                nc.any.tensor_copy(xT[:], xT_ps[:])

                f_ps = pacc.tile([P, D_OUT], f32, tag="pacc")
                for k in range(KC):
                    nc.tensor.matmul(f_ps[:], xT[:, k, :], wfx_f[:, k, :],
                                     start=(k == 0), stop=(k == KC - 1))
                out1 = work.tile([P, D_OUT], f32, tag="out1")
                nc.vector.tensor_tensor(out1[:], f_ps[:], bfxb_t[:], OP.add)
                nc.sync.dma_start(out=out_ap[ts(i, P), 0:D_OUT], in_=out1[:])

                xu_ps = pacc.tile([H, P], f32, tag="pacc")
                for k in range(KC):
                    nc.tensor.matmul(xu_ps[:], wx1_f[:, k, :], xT[:, k, :],
                                     start=(k == 0), stop=(k == KC - 1))
                txT = work.tile([H, P], f32, tag="txT")
                nc.scalar.activation(txT[:], xu_ps[:], AF.Tanh, bias=bx1_t[:, 0:1])

                xa_ps = pacc.tile([H, P], f32, tag="pacc")
                nc.tensor.matmul(xa_ps[:], wx2_f[:], txT[:], start=True, stop=True)
                nc.any.tensor_scalar(xattT[:, ts(i, P)], xa_ps[:], bx2_t[:, 0:1],
                                     None, OP.add)

            # value reload after phase 2: waits on the collectives anyway, and
            # only the (late) out2 writes sit behind it on the SP queue.
            # global tile for (core c, chunk g, j) is t = c*8 + 2g + j
            va_r = value_all[:].rearrange("p (c tl) w -> p c tl w", c=N_CORES)
            for g in range(4):
                for c in range(N_CORES):
                    src_ap = vb_gats[g].ap()[2 * c:2 * c + 2].rearrange(
                        "j p o -> p j o")
                    nc.sync.dma_start(
                        out=va_r[:, c, 2 * g:2 * g + 2, 0:D_OUT],
                        in_=src_ap)

            # ---------------- phase 3: fused masked attention ----------------
            for b in range(NB):
                adjf = adjf_list.pop(b)
                if b + 2 < NB:
                    adjf_list[b + 2] = blk.tile([P, N_NEIGH], bf16, tag="adjf",
                                            name=f"adjf{b+2}")
                    nc.gpsimd.dma_start(out=adjf_list[b + 2][:],
                                        in_=adj_in[ts(b + 2, P), :])
                scores = blk.tile([P, N_NEIGH], bf16, tag="scores")
                agg_ps = pacc.tile([P, VW], f32, tag="pacc")

                for c4 in range(4):            # 2048-wide chunks
                    for q in range(2):         # 1024-wide prelu chunks
                        ch = c4 * 2 + q        # 1024-chunk index (0..7)
                        s_ps = pws.tile([P, 1024], f32, tag="pws")
                        for hh in range(2):
                            c = ch * 2 + hh    # 512-chunk index (0..15)
                            nc.tensor.matmul(
                                s_ps[:, ts(hh, 512)], xattT[:, ts(b, P)],
                                nattT[:, c // 2, ds((c % 2) * 512, 512)],
                                start=True, stop=True)
                        nc.scalar.activation(scores[:, ts(ch, 1024)], s_ps[:],
                                             AF.Prelu, alpha=0.01)
                    nc.scalar.activation(scores[:, ts(c4, 2048)],
                                         scores[:, ts(c4, 2048)], AF.Exp)
                    nc.vector.tensor_tensor(scores[:, ts(c4, 2048)],
                                            scores[:, ts(c4, 2048)],
                                            adjf[:, ts(c4, 2048)], OP.mult)
                # (transposes/aggs moved below: consumption follows the
                # chunked value-gather arrival order)

                t_order = [c * NTL + 2 * g + j
                           for g in range(4) for c in range(N_CORES)
                           for j in range(2)]
                for bi in range(NT // TB):
                    batch = t_order[bi * TB:(bi + 1) * TB]
                    pT_ps = ppt.tile([P, TB, P], bf16, tag="ppt")
                    for i, t2 in enumerate(batch):
                        nc.tensor.transpose(pT_ps[:, i, :],
                                            scores[:, ts(t2, P)], ident_bf[:])
                    pT = work.tile([P, TB, P], bf16, tag="pT")
                    nc.vector.tensor_copy(pT[:], pT_ps[:])
                    for i, t2 in enumerate(batch):
                        nc.tensor.matmul(agg_ps[:], pT[:, i, :],
                                         value_all[:, t2, :],
                                         start=(bi == 0 and i == 0),
                                         stop=(bi == NT // TB - 1 and i == TB - 1))

                rcp = work.tile([P, 1], f32, tag="rcp")
                nc.vector.reciprocal(rcp[:], agg_ps[:, D_OUT:VW])
                out2 = work.tile([P, D_OUT], f32, tag="out2")
                nc.vector.tensor_scalar(out2[:], agg_ps[:, 0:D_OUT], rcp[:, 0:1],
                                        None, OP.mult)
                nc.vector.tensor_tensor(out2[:], out2[:], bvb_t[:], OP.add)
                nc.sync.dma_start(out=out_ap[ts(b, P), D_OUT:2 * D_OUT], in_=out2[:])

    nc.compile()
    return nc


def _get_nc():
    if "nc" not in _CACHE:
        _CACHE["nc"] = _build()
    return _CACHE["nc"]


def _prep_weights(Wx1, bx1, Wx2, bx2, Wn1, bn1, Wn2, bn2, Wv, bv, Wfx, bfx):
    def kstripe(w):  # (512, M) -> (128, 4, M) with ki on partitions
        m = w.shape[1]
        return np.ascontiguousarray(
            w.reshape(KC, P, m).transpose(1, 0, 2)).astype(np.float32)

    return {
        "wn1": kstripe(Wn1), "wx1": kstripe(Wx1),
        "wn2": np.ascontiguousarray(Wn2, np.float32),
        "wx2": np.ascontiguousarray(Wx2, np.float32),
        "wv": kstripe(Wv), "wfx": kstripe(Wfx),
        "bn1": np.ascontiguousarray(bn1, np.float32).reshape(H, 1),
        "bx1": np.ascontiguousarray(bx1, np.float32).reshape(H, 1),
        "bn2": np.ascontiguousarray(bn2, np.float32).reshape(H, 1),
        "bx2": np.ascontiguousarray(bx2, np.float32).reshape(H, 1),
        "bvb": np.ascontiguousarray(np.broadcast_to(bv, (P, D_OUT)), np.float32),
        "bfxb": np.ascontiguousarray(np.broadcast_to(bfx, (P, D_OUT)), np.float32),
    }


def make_in_maps(inputs):
    x = np.asarray(inputs["x"], np.float32)
    neigh = np.asarray(inputs["neigh"], np.float32)
    adj = np.asarray(inputs["adj"], np.int32)
    wmap = _prep_weights(*(np.asarray(inputs[k]) for k in
                           ("Wx1", "bx1", "Wx2", "bx2", "Wn1", "bn1", "Wn2", "bn2",
                            "Wv", "bv", "Wfx", "bfx")))
    in_maps = []
    for c in range(N_CORES):
        m = {"x": np.ascontiguousarray(x[c * R:(c + 1) * R]),
             "neigh_slice": np.ascontiguousarray(neigh[c * R:(c + 1) * R]),
             "adj": np.ascontiguousarray(adj[c * R:(c + 1) * R])}
        m.update(wmap)
        in_maps.append(m)
    return in_maps


def run(inputs, trace=False):
    nc = _get_nc()
    in_maps = make_in_maps(inputs)
    res = run_bass_kernel_spmd(nc, in_maps, list(range(N_CORES)), trace=trace)
    out = np.concatenate([res.results[c]["out"] for c in range(N_CORES)], axis=0)
    return out, res


def kernel(**inputs) -> np.ndarray:
    out, _ = run(inputs)
    return out
